# revision 1
# baseline (speedup 1.0000x reference)
"""Trainium2 Bass kernel for GroundwaterModel Jacobi pseudo-timestepping.

Solves 100 Jacobi steps of -div(exp(u) grad p) = f on a [1024,1024] grid,
sharded row-wise (x) across 8 NeuronCores with a 1-row halo exchange per
step (AllGather of pre-weighted boundary rows).

Math: with D = 2*eu + eu_xm + eu_ym (Jacobi diagonal), substitute
q = sqrt(D) * p.  The update becomes

  q'[i,j] = bx[i,j] q[i+1,j] + bx[i-1,j] q[i-1,j]
          + by[i,j] q[i,j+1] + by[i,j-1] q[i,j-1] + c[i,j]

with bx = eu/(s * s_up), by = eu/(s * s_yp), s = sqrt(D), c = h^2 f / s
(+ Dirichlet fold at the two y-boundary columns).  Every term is a pure
shift of an elementwise product, so the PE accumulates the whole update
into PSUM via shift/identity matmuls while the DVE only computes the four
products u1=bxd*q, u2=bx*q, u3=byd*q, u4=by*q.  Neumann x-edges are folded
into per-core shift-matrix corner entries; the inter-core halo is the
exchange of u1[row 0] (up) and u2[row 127] (down), selected from the
AllGather result by a per-core K=16 selection matmul.
"""

import numpy as np

GRID = 1024
NCORES = 8
P = 128          # rows per core = SBUF partitions
W = GRID - 2     # computed interior columns j=1..GRID-2
TS = 100

_cached = {}


def _host_inputs(u, f, n_cores, time_steps):
    """Per-core input dicts. All per-core variation lives in data."""
    N = u.shape[0]
    h = 1.0 / (N - 1)
    rows = N // n_cores
    xs = (np.arange(N, dtype=np.float64) * h).astype(np.float32)

    def clip_rows(lo):
        idx = np.clip(np.arange(lo, lo + rows), 0, N - 1)
        return u[idx].astype(np.float32)

    in_maps = []
    for c in range(n_cores):
        r0 = c * rows
        u0 = u[r0:r0 + rows].astype(np.float32)
        uu = clip_rows(r0 + 1)
        ud = clip_rows(r0 - 1)
        udd = clip_rows(r0 - 2)
        umid = u0.copy()
        if c == n_cores - 1:
            umid[-1] = u[N - 2]          # so denom_up[last] == denom[N-1]
        f0 = f[r0:r0 + rows].astype(np.float32)
        bc0 = xs[r0:r0 + rows].reshape(rows, 1).copy()
        bc1 = (1.0 - xs[r0:r0 + rows]).reshape(rows, 1).copy()

        sup = np.zeros((rows, rows), dtype=np.float32)
        for i in range(rows - 1):
            sup[i, i + 1] = 1.0
        if c == 0:
            sup[0, 0] = 1.0              # Neumann bottom edge via u1[0]
        sdn = np.zeros((rows, rows), dtype=np.float32)
        for i in range(1, rows):
            sdn[i, i - 1] = 1.0
        if c == n_cores - 1:
            sdn[rows - 1, rows - 1] = 1.0  # Neumann top edge via u2[last]

        et = np.zeros((2 * n_cores, rows), dtype=np.float32)
        if c > 0:
            et[2 * c - 1, 0] = 1.0       # prev core's u2[last] -> my row 0
        if c < n_cores - 1:
            et[2 * c + 2, rows - 1] = 1.0  # next core's u1[0] -> my last row
        in_maps.append({
            "u0": u0, "uu": uu, "ud": ud, "udd": udd, "umid": umid,
            "f0": f0, "bc0": bc0, "bc1": bc1,
            "supT": sup.T.copy(), "sdnT": sdn.T.copy(), "eT": et,
        })
    return in_maps


def _build(n_cores, time_steps, nx, ny):
    import concourse.bass as bass
    import concourse.bacc as bacc
    import concourse.mybir as mybir
    from concourse.tile import TileContext

    f32 = mybir.dt.float32
    f32r = mybir.dt.float32r
    AF = mybir.ActivationFunctionType
    OP = mybir.AluOpType
    G = ny
    Wl = G - 2
    h = 1.0 / (nx - 1)
    rows = nx // n_cores
    GR = 2 * n_cores                     # gathered rows

    nc = bacc.Bacc(
        "TRN2",
        target_bir_lowering=False,
        debug=False,
        num_devices=n_cores,
    )
    dp = nc.declare_dram_parameter
    u0_d = dp("u0", [rows, G], f32, isOutput=False)
    uu_d = dp("uu", [rows, G], f32, isOutput=False)
    ud_d = dp("ud", [rows, G], f32, isOutput=False)
    udd_d = dp("udd", [rows, G], f32, isOutput=False)
    umid_d = dp("umid", [rows, G], f32, isOutput=False)
    f0_d = dp("f0", [rows, G], f32, isOutput=False)
    bc0_d = dp("bc0", [rows, 1], f32, isOutput=False)
    bc1_d = dp("bc1", [rows, 1], f32, isOutput=False)
    supT_d = dp("supT", [rows, rows], f32, isOutput=False)
    sdnT_d = dp("sdnT", [rows, rows], f32, isOutput=False)
    eT_d = dp("eT", [GR, rows], f32, isOutput=False)
    pout_d = dp("pout", [rows, Wl], f32, isOutput=True)

    with TileContext(nc) as tc:
        with (
            tc.tile_pool(name="coef", bufs=1) as coef,
            tc.tile_pool(name="wts", bufs=1) as wts,
            tc.tile_pool(name="work", bufs=2) as work,
            tc.tile_pool(name="qp", bufs=2, space="PSUM") as qp,
            tc.tile_pool(name="dramp", bufs=2, space="DRAM") as dramp,
        ):
            # ---- persistent tiles ----
            bx = coef.tile([rows, Wl], f32, name="bx")
            bxd = coef.tile([rows, Wl], f32, name="bxd")
            by = coef.tile([rows, Wl], f32, name="by")
            cp = coef.tile([rows, Wl], f32, name="cp")
            cp0 = coef.tile([rows, Wl], f32, name="cp0")
            rs = coef.tile([rows, G], f32, name="rs")
            supT = wts.tile([rows, rows], f32, name="supT_t")
            sdnT = wts.tile([rows, rows], f32, name="sdnT_t")
            eT = wts.tile([GR, rows], f32, name="eT_t")
            nc.sync.dma_start(out=supT[:, :], in_=supT_d[:, :])
            nc.sync.dma_start(out=sdnT[:, :], in_=sdnT_d[:, :])
            nc.sync.dma_start(out=eT[:, :], in_=eT_d[:, :])

            # ---- setup: coefficients ----
            with tc.tile_pool(name="setup", bufs=1) as sp:
                u0 = sp.tile([rows, G], f32, name="u0_t")
                uu = sp.tile([rows, G], f32, name="uu_t")
                ud = sp.tile([rows, G], f32, name="ud_t")
                udd = sp.tile([rows, G], f32, name="udd_t")
                umid = sp.tile([rows, G], f32, name="umid_t")
                f0 = sp.tile([rows, G], f32, name="f0_t")
                bc0 = sp.tile([rows, 1], f32, name="bc0_t")
                bc1 = sp.tile([rows, 1], f32, name="bc1_t")
                nc.sync.dma_start(out=u0[:, :], in_=u0_d[:, :])
                nc.sync.dma_start(out=uu[:, :], in_=uu_d[:, :])
                nc.sync.dma_start(out=ud[:, :], in_=ud_d[:, :])
                nc.sync.dma_start(out=udd[:, :], in_=udd_d[:, :])
                nc.sync.dma_start(out=umid[:, :], in_=umid_d[:, :])
                nc.sync.dma_start(out=f0[:, :], in_=f0_d[:, :])
                nc.sync.dma_start(out=bc0[:, :], in_=bc0_d[:, :])
                nc.sync.dma_start(out=bc1[:, :], in_=bc1_d[:, :])

                eu = sp.tile([rows, G], f32, name="eu")
                eu_u = sp.tile([rows, G], f32, name="eu_u")
                eu_d = sp.tile([rows, G], f32, name="eu_d")
                eu_dd = sp.tile([rows, G], f32, name="eu_dd")
                eu_m = sp.tile([rows, G], f32, name="eu_m")
                nc.scalar.activation(eu[:, :], u0[:, :], AF.Exp)
                nc.scalar.activation(eu_u[:, :], uu[:, :], AF.Exp)
                nc.scalar.activation(eu_d[:, :], ud[:, :], AF.Exp)
                nc.scalar.activation(eu_dd[:, :], udd[:, :], AF.Exp)
                nc.scalar.activation(eu_m[:, :], umid[:, :], AF.Exp)

                den_s = sp.tile([rows, G], f32, name="den_s")
                den = sp.tile([rows, G], f32, name="den")
                dup_s = sp.tile([rows, G], f32, name="dup_s")
                dup = sp.tile([rows, G], f32, name="dup")
                ddn_s = sp.tile([rows, G], f32, name="ddn_s")
                ddn = sp.tile([rows, G], f32, name="ddn")
                V = nc.vector
                # denom cols 1..G-1
                V.scalar_tensor_tensor(den_s[:, 1:G], eu[:, 1:G], 2.0,
                                       eu_d[:, 1:G], OP.mult, OP.add)
                V.tensor_add(den[:, 1:G], den_s[:, 1:G], eu[:, 0:G - 1])
                V.scalar_tensor_tensor(dup_s[:, 1:G], eu_u[:, 1:G], 2.0,
                                       eu_m[:, 1:G], OP.mult, OP.add)
                V.tensor_add(dup[:, 1:G], dup_s[:, 1:G], eu_u[:, 0:G - 1])
                V.scalar_tensor_tensor(ddn_s[:, 1:G], eu_d[:, 1:G], 2.0,
                                       eu_dd[:, 1:G], OP.mult, OP.add)
                V.tensor_add(ddn[:, 1:G], ddn_s[:, 1:G], eu_d[:, 0:G - 1])

                rs_up = sp.tile([rows, G], f32, name="rs_up")
                rs_dn = sp.tile([rows, G], f32, name="rs_dn")
                nt_a = sp.tile([rows, G], f32, name="nt_a")
                nt_b = sp.tile([rows, G], f32, name="nt_b")

                def rsqrt_ref(out_ap, x_ap):
                    # ACT Sqrt seed + reciprocal, then 2 Newton iterations
                    # y' = y*(1.5 - 0.5*x*y^2) in fp32 on DVE.
                    nc.scalar.activation(nt_a[:, 1:G], x_ap, AF.Sqrt)
                    nc.vector.reciprocal(out_ap, nt_a[:, 1:G])
                    for _ in range(2):
                        nc.vector.tensor_mul(nt_a[:, 1:G], out_ap, out_ap)
                        nc.vector.tensor_mul(nt_b[:, 1:G], nt_a[:, 1:G], x_ap)
                        nc.vector.tensor_scalar(nt_a[:, 1:G], nt_b[:, 1:G],
                                                -0.5, 1.5, OP.mult, OP.add)
                        nc.vector.tensor_mul(nt_b[:, 1:G], out_ap,
                                             nt_a[:, 1:G])
                        nc.vector.tensor_copy(out_ap, nt_b[:, 1:G])
                    return out_ap

                rsqrt_ref(rs[:, 1:G], den[:, 1:G])
                rsqrt_ref(rs_up[:, 1:G], dup[:, 1:G])
                rsqrt_ref(rs_dn[:, 1:G], ddn[:, 1:G])

                t1 = sp.tile([rows, Wl], f32, name="t1")
                # bx[k] = eu[k+1]*rs[k+1]*rs_up[k+1]  (tile col k == global j=k+1)
                V.tensor_mul(t1[:, :], eu[:, 1:1 + Wl], rs[:, 1:1 + Wl])
                V.tensor_mul(bx[:, :], t1[:, :], rs_up[:, 1:1 + Wl])
                # bxd[k] = eu_d[k+1]*rs_dn[k+1]*rs[k+1]
                V.tensor_mul(t1[:, :], eu_d[:, 1:1 + Wl], rs_dn[:, 1:1 + Wl])
                V.tensor_mul(bxd[:, :], t1[:, :], rs[:, 1:1 + Wl])
                # by[k] = eu[k+1]*rs[k+1]*rs[k+2]
                V.tensor_mul(t1[:, :], eu[:, 1:1 + Wl], rs[:, 2:2 + Wl])
                V.tensor_mul(by[:, :], t1[:, :], rs[:, 1:1 + Wl])

                # cp: h2f*rs with Dirichlet fold at cols 0 and Wl-1
                h2f = sp.tile([rows, G], f32, name="h2f")
                V.tensor_scalar_mul(h2f[:, :], f0[:, :], h * h)
                e0 = sp.tile([rows, 1], f32, name="e0")
                e1 = sp.tile([rows, 1], f32, name="e1")
                V.scalar_tensor_tensor(e0[:, :], eu[:, 0:1], bc0[:, :],
                                       h2f[:, 1:2], OP.mult, OP.add)
                V.scalar_tensor_tensor(e1[:, :], eu[:, G - 2:G - 1], bc1[:, :],
                                       h2f[:, G - 2:G - 1], OP.mult, OP.add)
                V.tensor_mul(cp[:, 1:Wl - 1], h2f[:, 2:G - 2], rs[:, 2:G - 2])
                V.tensor_mul(cp[:, 0:1], e0[:, :], rs[:, 1:2])
                V.tensor_mul(cp[:, Wl - 1:Wl], e1[:, :], rs[:, G - 2:G - 1])
                # q_1 uses the unfolded constant (reference's p0 has zero BCs)
                V.tensor_mul(cp0[:, :], h2f[:, 1:G - 1], rs[:, 1:G - 1])

            # ---- iteration ----
            B0 = 512                     # PSUM bank split
            banks = [(0, min(B0, Wl))] + ([(B0, Wl)] if Wl > B0 else [])
            rg = [list(range(n_cores))]
            V = nc.vector
            mm = nc.tensor.matmul
            qf = None
            u1 = u2 = gsb = None
            for t in range(1, time_steps + 1):
                if t == 1:
                    qf = work.tile([rows, Wl], f32, tag="qf", name="qf_1")
                    V.tensor_copy(qf[:, :], cp0[:, :])  # q_1 = h2f*rs
                else:
                    # PE: x-shift terms + halo into PSUM
                    ps = qp.tile([rows, Wl], f32, tag="ps", name=f"ps_{t}")
                    for lo, hi in banks:
                        mm(ps[:, lo:hi], supT[:, :], u1[:, lo:hi],
                           start=True, stop=False)
                        mm(ps[:, lo:hi], sdnT[:, :], u2[:, lo:hi],
                           start=False, stop=False)
                        mm(ps[:, lo:hi], eT[:, :], gsb[:, lo:hi],
                           start=False, stop=True)
                    # DVE: y-terms (free-dim shifted reads of qf_prev)
                    y3 = work.tile([rows, Wl], f32, tag="y3", name=f"y3_{t}")
                    y4 = work.tile([rows, Wl], f32, tag="y4", name=f"y4_{t}")
                    V.memset(y3[:, Wl - 1:Wl], 0.0)
                    V.memset(y4[:, 0:1], 0.0)
                    V.tensor_mul(y3[:, 0:Wl - 1], by[:, 0:Wl - 1],
                                 qf[:, 1:Wl])
                    V.tensor_mul(y4[:, 1:Wl], by[:, 0:Wl - 1],
                                 qf[:, 0:Wl - 1])
                    a1 = work.tile([rows, Wl], f32, tag="a1", name=f"a1_{t}")
                    V.tensor_add(a1[:, :], y3[:, :], y4[:, :])
                    a2 = work.tile([rows, Wl], f32, tag="a2", name=f"a2_{t}")
                    V.tensor_add(a2[:, :], a1[:, :], cp[:, :])
                    qf = work.tile([rows, Wl], f32, tag="qf", name=f"qf_{t}")
                    V.tensor_add(qf[:, :], a2[:, :], ps[:, :])

                if t < time_steps:
                    u1 = work.tile([rows, Wl], f32, tag="u1", name=f"u1_{t}")
                    u2 = work.tile([rows, Wl], f32, tag="u2", name=f"u2_{t}")
                    V.tensor_mul(u1[:, :], bxd[:, :], qf[:, :])
                    V.tensor_mul(u2[:, :], bx[:, :], qf[:, :])
                    bounce = dramp.tile([2, Wl], f32, tag="bounce",
                                        name=f"bounce_{t}")
                    gath = dramp.tile([GR, Wl], f32, tag="gath",
                                      addr_space="Shared", name=f"gath_{t}")
                    nc.sync.dma_start(out=bounce[0:1, :], in_=u1[0:1, :])
                    nc.sync.dma_start(out=bounce[1:2, :],
                                      in_=u2[rows - 1:rows, :])
                    nc.gpsimd.collective_compute(
                        "AllGather", OP.bypass,
                        ins=[bounce.opt()], outs=[gath.opt()],
                        replica_groups=rg,
                    )
                    gsb = work.tile([GR, Wl], f32, tag="gsb", name=f"gsb_{t}")
                    nc.sync.dma_start(out=gsb[:, :], in_=gath[:, :])

            out_sb = coef.tile([rows, Wl], f32, name="out_sb")
            nc.vector.tensor_mul(out_sb[:, :], qf[:, :], rs[:, 1:1 + Wl])
            nc.sync.dma_start(out=pout_d[:, :], in_=out_sb[:, :])

    nc.finalize()
    return nc


def _get_nc(n_cores, time_steps, nx, ny):
    key = (n_cores, time_steps, nx, ny)
    if key not in _cached:
        _cached[key] = _build(n_cores, time_steps, nx, ny)
    return _cached[key]


def kernel(u, f, time_steps):
    from concourse.bass_utils import run_bass_kernel_spmd

    u = np.asarray(u)
    f = np.asarray(f)
    ts = int(time_steps)
    N = u.shape[0]
    n_cores = NCORES
    nc = _get_nc(n_cores, ts, N, u.shape[1])
    in_maps = _host_inputs(u, f, n_cores, ts)
    res = run_bass_kernel_spmd(nc, in_maps, list(range(n_cores))).results
    interior = np.concatenate([r["pout"] for r in res], axis=0)
    h = 1.0 / (N - 1)
    xs = (np.arange(N, dtype=np.float64) * h).astype(np.float32)
    out = np.empty((N, N), dtype=np.float32)
    out[:, 1:N - 1] = interior
    out[:, 0] = xs
    out[:, N - 1] = 1.0 - xs
    return out



# revision 3
# speedup vs baseline: 1.5549x; 1.5549x over previous
"""Trainium2 Bass kernel for GroundwaterModel Jacobi pseudo-timestepping.

Solves 100 Jacobi steps of -div(exp(u) grad p) = f on a [1024,1024] grid,
sharded row-wise (x) across 8 NeuronCores with a 1-row halo exchange per
step (AllGather of pre-weighted boundary rows).

Math: with D = 2*eu + eu_xm + eu_ym (Jacobi diagonal), substitute
q = lam * sqrt(D) * p.  The update becomes

  q'[i,k] = bxu[i,k] q[i+1,k] + bxu[i-1,k] q[i-1,k]
          + by[i,k] q[i,k+1] + by[i,k-1] q[i,k-1] + c[i,k]

with bxu[i,k] = eu[i,j]*rs[i,j]*rs[i+1,j], by[i,k] = eu[i,j]*rs[i,j]*rs[i,j+1],
rs = 1/sqrt(D), c = lam*h^2*f*rs (+ Dirichlet fold at the two y-boundary
columns, Neumann folds at the x edges).  All coefficients are precomputed
on the host in fp64 and shipped as fp16; the iteration state q and the four
shift products run in fp16 on the DVE (2x mode), the partition-dim shifts
and halo injection accumulate in fp32 PSUM via fp16 matmuls (1 cycle/row),
and the per-step inter-core halo is an AllGather of the two boundary
products, issued at the top of each step so it overlaps the interior work.
"""

import numpy as np

GRID = 1024
NCORES = 8
P = 128          # rows per core = SBUF partitions
W = GRID - 2     # computed interior columns j=1..GRID-2
LAM = 1024.0     # q scaling to keep fp16 constants out of the subnormal range
CC_MODE = "ag8"  # "ag8": one 8-core AllGather; "pair": two 2-core AllGathers

_cached = {}


def _host_inputs(u, f, n_cores, time_steps, cc_mode=CC_MODE):
    """Per-core input dicts. All per-core variation lives in data."""
    N = u.shape[0]
    h = 1.0 / (N - 1)
    rows = N // n_cores
    Wl = N - 2

    eu = np.exp(u.astype(np.float64))
    eu_xm = np.concatenate([eu[:1, :], eu[:-1, :]], axis=0)
    eu_ym = np.concatenate([eu[:, :1], eu[:, :-1]], axis=1)
    D = 2.0 * eu + eu_xm + eu_ym
    rs = 1.0 / np.sqrt(D)
    h2f = (h * h) * f.astype(np.float64)
    xs = np.arange(N, dtype=np.float64) * h
    bc0 = xs
    bc1 = 1.0 - xs
    j = np.arange(1, N - 1)

    # x-coupling (i,j)<->(i+1,j); row N-1 replaced by the Neumann bottom fold
    bxu = np.zeros((N, Wl))
    bxu[:-1, :] = eu[:-1, j] * rs[:-1, j] * rs[1:, j]
    b_top = eu[0, j] * rs[0, j] * rs[0, j]
    b_bot = eu[N - 1, j] * rs[N - 1, j] * rs[N - 1, j]
    # y-coupling (i,j)<->(i,j+1); column Wl-1 is Dirichlet-folded -> 0
    by = np.zeros((N, Wl))
    by[:, :-1] = eu[:, j[:-1]] * rs[:, j[:-1]] * rs[:, j[:-1] + 1]
    # constants
    c0 = h2f[:, j] * rs[:, j]
    c = c0.copy()
    c[:, 0] += eu_ym[:, 1] * bc0 * rs[:, 1]
    c[:, -1] += eu[:, N - 2] * bc1 * rs[:, N - 2]
    c = LAM * c
    c0 = LAM * c0

    f16 = np.float16
    in_maps = []
    for cidx in range(n_cores):
        r0 = cidx * rows
        A = bxu[r0:r0 + rows].copy()
        if cidx == n_cores - 1:
            A[-1] = b_bot
        Ad = np.zeros((rows, Wl))
        Ad[1:] = bxu[r0:r0 + rows - 1]
        Ad[0] = b_top if cidx == 0 else bxu[r0 - 1]
        By = by[r0:r0 + rows]
        Byd = np.zeros((rows, Wl))
        Byd[:, 1:] = By[:, :-1]

        supT = np.zeros((rows, rows), dtype=f16)
        for i in range(rows - 1):
            supT[i + 1, i] = 1.0          # out[i] += u1[i+1]
        if cidx == 0:
            supT[0, 0] = 1.0              # Neumann top edge via u1[0]
        sdnT = np.zeros((rows, rows), dtype=f16)
        for i in range(1, rows):
            sdnT[i - 1, i] = 1.0          # out[i] += u2[i-1]
        if cidx == n_cores - 1:
            sdnT[rows - 1, rows - 1] = 1.0  # Neumann bottom edge via u2[last]

        if cc_mode == "ag8":
            GR = 2 * n_cores
            eT = np.zeros((GR, rows), dtype=f16)
            if cidx > 0:
                eT[2 * cidx - 1, 0] = 1.0       # prev core's tx2 -> my row 0
            if cidx < n_cores - 1:
                eT[2 * cidx + 2, rows - 1] = 1.0  # next core's tx1 -> my last row
        else:
            # two 2-core AllGathers; gsb rows 0-3 = CC1 pair, 4-7 = CC2 pair,
            # each pair in ascending rank order as [lo_tx1, lo_tx2, hi_tx1, hi_tx2]
            eT = np.zeros((8, rows), dtype=f16)
            if cidx % 2 == 0:
                if cidx + 1 < n_cores:
                    eT[2, rows - 1] = 1.0   # CC1 partner is next: its tx1
                if cidx > 0:
                    eT[4 + 1, 0] = 1.0      # CC2 partner is prev: its tx2
            else:
                eT[1, 0] = 1.0              # CC1 partner is prev: its tx2
                if cidx + 1 < n_cores:
                    eT[4 + 2, rows - 1] = 1.0  # CC2 partner is next: its tx1

        in_maps.append({
            "A": A.astype(f16), "Ad": Ad.astype(f16),
            "By": By.astype(f16), "Byd": Byd.astype(f16),
            "cp": c[r0:r0 + rows].astype(f16),
            "cp0": c0[r0:r0 + rows].astype(f16),
            "supT": supT, "sdnT": sdnT, "eT": eT,
            "rsl": (rs[r0:r0 + rows, j] / LAM).astype(np.float32),
        })
    return in_maps


def _build(n_cores, time_steps, nx, ny, cc_mode=CC_MODE):
    import concourse.bass as bass
    import concourse.bacc as bacc
    import concourse.mybir as mybir
    from concourse.tile import TileContext

    f32 = mybir.dt.float32
    f16 = mybir.dt.float16
    G = ny
    Wl = G - 2
    rows = nx // n_cores
    GR = 2 * n_cores if cc_mode == "ag8" else 8

    nc = bacc.Bacc(
        "TRN2",
        target_bir_lowering=False,
        debug=False,
        num_devices=n_cores,
    )
    dp = nc.declare_dram_parameter
    A_d = dp("A", [rows, Wl], f16, isOutput=False)
    Ad_d = dp("Ad", [rows, Wl], f16, isOutput=False)
    By_d = dp("By", [rows, Wl], f16, isOutput=False)
    Byd_d = dp("Byd", [rows, Wl], f16, isOutput=False)
    cp_d = dp("cp", [rows, Wl], f16, isOutput=False)
    cp0_d = dp("cp0", [rows, Wl], f16, isOutput=False)
    supT_d = dp("supT", [rows, rows], f16, isOutput=False)
    sdnT_d = dp("sdnT", [rows, rows], f16, isOutput=False)
    eT_d = dp("eT", [GR, rows], f16, isOutput=False)
    rsl_d = dp("rsl", [rows, Wl], f32, isOutput=False)
    pout_d = dp("pout", [rows, Wl], f32, isOutput=True)

    if cc_mode == "ag8":
        rg_list = [[list(range(n_cores))]]
    else:
        rg_list = [
            [[0, 1], [2, 3], [4, 5], [6, 7]],
            [[0, 7], [1, 2], [3, 4], [5, 6]],
        ]

    with TileContext(nc) as tc:
        with (
            tc.tile_pool(name="coef", bufs=1) as coef,
            tc.tile_pool(name="work", bufs=2) as work,
            tc.tile_pool(name="qp", bufs=2, space="PSUM") as qp,
            tc.tile_pool(name="dramp", bufs=2, space="DRAM") as dramp,
        ):
            # ---- persistent tiles (coefficients + state) ----
            A = coef.tile([rows, Wl], f16, name="A_t")
            Ad = coef.tile([rows, Wl], f16, name="Ad_t")
            By = coef.tile([rows, Wl], f16, name="By_t")
            Byd = coef.tile([rows, Wl], f16, name="Byd_t")
            cp = coef.tile([rows, Wl], f16, name="cp_t")
            cp0 = coef.tile([rows, Wl], f16, name="cp0_t")
            supT = coef.tile([rows, rows], f16, name="supT_t")
            sdnT = coef.tile([rows, rows], f16, name="sdnT_t")
            eT = coef.tile([GR, rows], f16, name="eT_t")
            rsl = coef.tile([rows, Wl], f32, name="rsl_t")
            qe = coef.tile([rows, Wl + 2], f16, name="qe")  # pad cols 0, Wl+1
            for t_, d_ in ((A, A_d), (Ad, Ad_d), (By, By_d), (Byd, Byd_d),
                           (cp, cp_d), (cp0, cp0_d), (supT, supT_d),
                           (sdnT, sdnT_d), (eT, eT_d), (rsl, rsl_d)):
                nc.sync.dma_start(out=t_[:, :], in_=d_[:, :])

            B0 = 512                     # PSUM bank split
            banks = [(0, B0), (B0, Wl)]
            V = nc.vector
            mm = nc.tensor.matmul

            nc.vector.memset(qe[:, 0:1], 0.0)
            nc.vector.memset(qe[:, Wl + 1:Wl + 2], 0.0)

            gsb = None
            for t in range(1, time_steps + 1):
                if t == 1:
                    V.tensor_copy(qe[:, 1:Wl + 1], cp0[:, :])  # q_1 = lam*h2f*rs
                else:
                    # products for the x-shifts (PE) and y-shifts (free-dim)
                    u1 = work.tile([rows, Wl], f16, tag="u1", name=f"u1_{t}")
                    u2 = work.tile([rows, Wl], f16, tag="u2", name=f"u2_{t}")
                    y1 = work.tile([rows, Wl], f16, tag="y1", name=f"y1_{t}")
                    y2 = work.tile([rows, Wl], f16, tag="y2", name=f"y2_{t}")
                    V.tensor_mul(u1[:, :], Ad[:, :], qe[:, 1:Wl + 1])
                    V.tensor_mul(u2[:, :], A[:, :], qe[:, 1:Wl + 1])
                    V.tensor_mul(y1[:, :], By[:, :], qe[:, 2:Wl + 2])
                    V.tensor_mul(y2[:, :], Byd[:, :], qe[:, 0:Wl])
                    # PE: x-shift terms + halo into PSUM (eT last: waits on CC)
                    ps = qp.tile([rows, Wl], f32, tag="ps", name=f"ps_{t}")
                    for lo, hi in banks:
                        mm(ps[:, lo:hi], supT[:, :], u1[:, lo:hi],
                           start=True, stop=False)
                    for lo, hi in banks:
                        mm(ps[:, lo:hi], sdnT[:, :], u2[:, lo:hi],
                           start=False, stop=False)
                    for lo, hi in banks:
                        mm(ps[:, lo:hi], eT[:, :], gsb[:, lo:hi],
                           start=False, stop=True)
                    a1 = work.tile([rows, Wl], f16, tag="a1", name=f"a1_{t}")
                    a2 = work.tile([rows, Wl], f16, tag="a2", name=f"a2_{t}")
                    V.tensor_add(a1[:, :], y1[:, :], y2[:, :])
                    V.tensor_add(a2[:, :], a1[:, :], cp[:, :])
                    V.tensor_add(qe[:, 1:Wl + 1], a2[:, :], ps[:, :])

                if t < time_steps:
                    # boundary products only -> bounce -> AllGather, issued
                    # before the next step's interior work so the collective
                    # overlaps it
                    # quadrant-aligned (partition base must be 0/32/64/96);
                    # only rows 0 and 127 are consumed by the bounce DMA
                    tx = work.tile([rows, Wl], f16, tag="tx", name=f"tx_{t}")
                    V.tensor_mul(tx[0:32, :], Ad[0:32, :], qe[0:32, 1:Wl + 1])
                    V.tensor_mul(tx[96:128, :], A[96:128, :],
                                 qe[96:128, 1:Wl + 1])
                    bounce = dramp.tile([2, Wl], f16, tag="bounce",
                                        name=f"bounce_{t}")
                    nc.sync.dma_start(out=bounce[0:1, :], in_=tx[0:1, :])
                    nc.sync.dma_start(out=bounce[1:2, :],
                                      in_=tx[rows - 1:rows, :])
                    gsb = work.tile([GR, Wl], f16, tag="gsb", name=f"gsb_{t}")
                    for gi, rg in enumerate(rg_list):
                        gw = 2 * len(rg[0])
                        gath = dramp.tile([gw, Wl], f16, tag=f"gath{gi}",
                                          addr_space="Shared",
                                          name=f"gath{gi}_{t}")
                        nc.gpsimd.collective_compute(
                            "AllGather", mybir.AluOpType.bypass,
                            ins=[bounce.opt()], outs=[gath.opt()],
                            replica_groups=rg,
                        )
                        nc.sync.dma_start(out=gsb[gi * gw:(gi + 1) * gw, :],
                                          in_=gath[:, :])

            out_sb = coef.tile([rows, Wl], f32, name="out_sb")
            nc.vector.tensor_mul(out_sb[:, :], qe[:, 1:Wl + 1], rsl[:, :])
            nc.sync.dma_start(out=pout_d[:, :], in_=out_sb[:, :])

    nc.finalize()
    return nc


def _get_nc(n_cores, time_steps, nx, ny):
    key = (n_cores, time_steps, nx, ny)
    if key not in _cached:
        _cached[key] = _build(n_cores, time_steps, nx, ny)
    return _cached[key]


def kernel(u, f, time_steps):
    from concourse.bass_utils import run_bass_kernel_spmd

    u = np.asarray(u)
    f = np.asarray(f)
    ts = int(time_steps)
    N = u.shape[0]
    n_cores = NCORES
    nc = _get_nc(n_cores, ts, N, u.shape[1])
    in_maps = _host_inputs(u, f, n_cores, ts)
    res = run_bass_kernel_spmd(nc, in_maps, list(range(n_cores))).results
    interior = np.concatenate([r["pout"] for r in res], axis=0)
    h = 1.0 / (N - 1)
    xs = (np.arange(N, dtype=np.float64) * h).astype(np.float32)
    out = np.empty((N, N), dtype=np.float32)
    out[:, 1:N - 1] = interior
    out[:, 0] = xs
    out[:, N - 1] = 1.0 - xs
    return out


# revision 13
# speedup vs baseline: 1.5677x; 1.0082x over previous
"""Trainium2 Bass kernel for GroundwaterModel Jacobi pseudo-timestepping.

Solves 100 Jacobi steps of -div(exp(u) grad p) = f on a [1024,1024] grid,
sharded row-wise (x) across 8 NeuronCores with a 1-row halo exchange per
step (AllGather of pre-weighted boundary rows).

Math: with D = 2*eu + eu_xm + eu_ym (Jacobi diagonal), substitute
q = lam * sqrt(D) * p.  The update becomes

  q'[i,k] = bxu[i,k] q[i+1,k] + bxu[i-1,k] q[i-1,k]
          + by[i,k] q[i,k+1] + by[i,k-1] q[i,k-1] + c[i,k]

with bxu[i,k] = eu[i,j]*rs[i,j]*rs[i+1,j], by[i,k] = eu[i,j]*rs[i,j]*rs[i,j+1],
rs = 1/sqrt(D), c = lam*h^2*f*rs (+ Dirichlet fold at the two y-boundary
columns, Neumann folds at the x edges).  All coefficients are precomputed
on the host in fp64 and shipped as fp16; the iteration state q and the four
shift products run in fp16 on the DVE (2x mode), the partition-dim shifts
and halo injection accumulate in fp32 PSUM via fp16 matmuls (1 cycle/row),
and the per-step inter-core halo is an AllGather of the two boundary
products, issued at the top of each step so it overlaps the interior work.
"""

import numpy as np

GRID = 1024
NCORES = 8
P = 128          # rows per core = SBUF partitions
W = GRID - 2     # computed interior columns j=1..GRID-2
LAM = 1024.0     # q scaling to keep fp16 constants out of the subnormal range
CC_MODE = "ag8"  # "ag8": one 8-core AllGather; "pair": two 2-core AllGathers (hangs NRT)

_cached = {}


def _host_inputs(u, f, n_cores, time_steps, cc_mode=CC_MODE):
    """Per-core input dicts. All per-core variation lives in data."""
    N = u.shape[0]
    h = 1.0 / (N - 1)
    rows = N // n_cores
    Wl = N - 2

    eu = np.exp(u.astype(np.float64))
    eu_xm = np.concatenate([eu[:1, :], eu[:-1, :]], axis=0)
    eu_ym = np.concatenate([eu[:, :1], eu[:, :-1]], axis=1)
    D = 2.0 * eu + eu_xm + eu_ym
    rs = 1.0 / np.sqrt(D)
    h2f = (h * h) * f.astype(np.float64)
    xs = np.arange(N, dtype=np.float64) * h
    bc0 = xs
    bc1 = 1.0 - xs
    j = np.arange(1, N - 1)

    # x-coupling (i,j)<->(i+1,j); row N-1 replaced by the Neumann bottom fold
    bxu = np.zeros((N, Wl))
    bxu[:-1, :] = eu[:-1, j] * rs[:-1, j] * rs[1:, j]
    b_top = eu[0, j] * rs[0, j] * rs[0, j]
    b_bot = eu[N - 1, j] * rs[N - 1, j] * rs[N - 1, j]
    # y-coupling (i,j)<->(i,j+1); column Wl-1 is Dirichlet-folded -> 0
    by = np.zeros((N, Wl))
    by[:, :-1] = eu[:, j[:-1]] * rs[:, j[:-1]] * rs[:, j[:-1] + 1]
    # constants
    c0 = h2f[:, j] * rs[:, j]
    c = c0.copy()
    c[:, 0] += eu_ym[:, 1] * bc0 * rs[:, 1]
    c[:, -1] += eu[:, N - 2] * bc1 * rs[:, N - 2]
    c = LAM * c
    c0 = LAM * c0

    f16 = np.float16
    in_maps = []
    for cidx in range(n_cores):
        r0 = cidx * rows
        A = bxu[r0:r0 + rows].copy()
        if cidx == n_cores - 1:
            A[-1] = b_bot
        Ad = np.zeros((rows, Wl))
        Ad[1:] = bxu[r0:r0 + rows - 1]
        Ad[0] = b_top if cidx == 0 else bxu[r0 - 1]
        By = by[r0:r0 + rows]
        Byd = np.zeros((rows, Wl))
        Byd[:, 1:] = By[:, :-1]

        supT = np.zeros((rows, rows), dtype=f16)
        for i in range(rows - 1):
            supT[i + 1, i] = 1.0          # out[i] += u1[i+1]
        if cidx == 0:
            supT[0, 0] = 1.0              # Neumann top edge via u1[0]
        sdnT = np.zeros((rows, rows), dtype=f16)
        for i in range(1, rows):
            sdnT[i - 1, i] = 1.0          # out[i] += u2[i-1]
        if cidx == n_cores - 1:
            sdnT[rows - 1, rows - 1] = 1.0  # Neumann bottom edge via u2[last]

        if cc_mode == "ag8":
            GR = 2 * n_cores
            eT = np.zeros((GR, rows), dtype=f16)
            if cidx > 0:
                eT[2 * cidx - 1, 0] = 1.0       # prev core's tx2 -> my row 0
            if cidx < n_cores - 1:
                eT[2 * cidx + 2, rows - 1] = 1.0  # next core's tx1 -> my last row
        else:
            # two 2-core AllGathers; gsb rows 0-3 = CC1 pair, 4-7 = CC2 pair,
            # each pair in ascending rank order as [lo_tx1, lo_tx2, hi_tx1, hi_tx2]
            eT = np.zeros((8, rows), dtype=f16)
            if cidx % 2 == 0:
                if cidx + 1 < n_cores:
                    eT[2, rows - 1] = 1.0   # CC1 partner is next: its tx1
                if cidx > 0:
                    eT[4 + 1, 0] = 1.0      # CC2 partner is prev: its tx2
            else:
                eT[1, 0] = 1.0              # CC1 partner is prev: its tx2
                if cidx + 1 < n_cores:
                    eT[4 + 2, rows - 1] = 1.0  # CC2 partner is next: its tx1

        txc = np.zeros((rows, Wl))
        txc[0] = Ad[0]          # tx1: product sent to prev core
        txc[-1] = A[-1]         # tx2: product sent to next core
        in_maps.append({
            "A": A.astype(f16), "Ad": Ad.astype(f16), "txc": txc.astype(f16),
            "By": By.astype(f16), "Byd": Byd.astype(f16),
            "cp": c[r0:r0 + rows].astype(f16),
            "cp0": c0[r0:r0 + rows].astype(f16),
            "supT": supT, "sdnT": sdnT, "eT": eT,
            "rsl": (rs[r0:r0 + rows, j] / LAM).astype(np.float32),
        })
    return in_maps


def _build(n_cores, time_steps, nx, ny, cc_mode=CC_MODE):
    import concourse.bass as bass
    import concourse.bacc as bacc
    import concourse.mybir as mybir
    from concourse.tile import TileContext

    f32 = mybir.dt.float32
    f16 = mybir.dt.float16
    G = ny
    Wl = G - 2
    rows = nx // n_cores
    GR = 2 * n_cores if cc_mode == "ag8" else 8

    nc = bacc.Bacc(
        "TRN2",
        target_bir_lowering=False,
        debug=False,
        num_devices=n_cores,
    )
    dp = nc.declare_dram_parameter
    A_d = dp("A", [rows, Wl], f16, isOutput=False)
    Ad_d = dp("Ad", [rows, Wl], f16, isOutput=False)
    txc_d = dp("txc", [rows, Wl], f16, isOutput=False)
    By_d = dp("By", [rows, Wl], f16, isOutput=False)
    Byd_d = dp("Byd", [rows, Wl], f16, isOutput=False)
    cp_d = dp("cp", [rows, Wl], f16, isOutput=False)
    cp0_d = dp("cp0", [rows, Wl], f16, isOutput=False)
    supT_d = dp("supT", [rows, rows], f16, isOutput=False)
    sdnT_d = dp("sdnT", [rows, rows], f16, isOutput=False)
    eT_d = dp("eT", [GR, rows], f16, isOutput=False)
    rsl_d = dp("rsl", [rows, Wl], f32, isOutput=False)
    pout_d = dp("pout", [rows, Wl], f32, isOutput=True)

    if cc_mode == "ag8":
        rg_list = [[list(range(n_cores))]]
    else:
        rg_list = [
            [[0, 1], [2, 3], [4, 5], [6, 7]],
            [[0, 7], [1, 2], [3, 4], [5, 6]],
        ]

    with TileContext(nc) as tc:
        with (
            tc.tile_pool(name="coef", bufs=1) as coef,
            tc.tile_pool(name="work", bufs=2) as work,
            tc.tile_pool(name="qp", bufs=2, space="PSUM") as qp,
            tc.tile_pool(name="dramp", bufs=2, space="DRAM") as dramp,
        ):
            # ---- persistent tiles (coefficients + state) ----
            A = coef.tile([rows, Wl], f16, name="A_t")
            Ad = coef.tile([rows, Wl], f16, name="Ad_t")
            txc = coef.tile([rows, Wl], f16, name="txc_t")
            By = coef.tile([rows, Wl], f16, name="By_t")
            Byd = coef.tile([rows, Wl], f16, name="Byd_t")
            cp = coef.tile([rows, Wl], f16, name="cp_t")
            cp0 = coef.tile([rows, Wl], f16, name="cp0_t")
            supT = coef.tile([rows, rows], f16, name="supT_t")
            sdnT = coef.tile([rows, rows], f16, name="sdnT_t")
            eT = coef.tile([GR, rows], f16, name="eT_t")
            rsl = coef.tile([rows, Wl], f32, name="rsl_t")
            qe = coef.tile([rows, Wl + 2], f16, name="qe")  # pad cols 0, Wl+1
            for t_, d_ in ((A, A_d), (Ad, Ad_d), (txc, txc_d),
                           (By, By_d), (Byd, Byd_d),
                           (cp, cp_d), (cp0, cp0_d), (supT, supT_d),
                           (sdnT, sdnT_d), (eT, eT_d), (rsl, rsl_d)):
                nc.sync.dma_start(out=t_[:, :], in_=d_[:, :])

            B0 = 512                     # PSUM bank split
            banks = [(0, B0), (B0, Wl)]
            V = nc.vector
            mm = nc.tensor.matmul

            nc.vector.memset(qe[:, 0:1], 0.0)
            nc.vector.memset(qe[:, Wl + 1:Wl + 2], 0.0)

            gsb = None
            for t in range(1, time_steps + 1):
                if t == 1:
                    V.tensor_copy(qe[:, 1:Wl + 1], cp0[:, :])  # q_1 = lam*h2f*rs
                else:
                    # products for the x-shifts (PE) and y-shifts (free-dim)
                    u1 = work.tile([rows, Wl], f16, tag="u1", name=f"u1_{t}")
                    u2 = work.tile([rows, Wl], f16, tag="u2", name=f"u2_{t}")
                    y1 = work.tile([rows, Wl], f16, tag="y1", name=f"y1_{t}")
                    y2 = work.tile([rows, Wl], f16, tag="y2", name=f"y2_{t}")
                    V.tensor_mul(u1[:, :], Ad[:, :], qe[:, 1:Wl + 1])
                    V.tensor_mul(u2[:, :], A[:, :], qe[:, 1:Wl + 1])
                    V.tensor_mul(y1[:, :], By[:, :], qe[:, 2:Wl + 2])
                    V.tensor_mul(y2[:, :], Byd[:, :], qe[:, 0:Wl])
                    # PE: x-shift terms + halo into PSUM (eT last: waits on CC)
                    ps = qp.tile([rows, Wl], f32, tag="ps", name=f"ps_{t}")
                    for lo, hi in banks:
                        mm(ps[:, lo:hi], supT[:, :], u1[:, lo:hi],
                           start=True, stop=False)
                    for lo, hi in banks:
                        mm(ps[:, lo:hi], sdnT[:, :], u2[:, lo:hi],
                           start=False, stop=False)
                    for lo, hi in banks:
                        mm(ps[:, lo:hi], eT[:, :], gsb[:, lo:hi],
                           start=False, stop=True)
                    a1 = work.tile([rows, Wl], f16, tag="a1", name=f"a1_{t}")
                    a2 = work.tile([rows, Wl], f16, tag="a2", name=f"a2_{t}")
                    V.tensor_add(a1[:, :], y1[:, :], y2[:, :])
                    V.tensor_add(a2[:, :], a1[:, :], cp[:, :])
                    V.tensor_add(qe[:, 1:Wl + 1], a2[:, :], ps[:, :])

                if t < time_steps:
                    # merged boundary product (txc is zero outside rows
                    # 0/127), one partition-strided bounce DMA; DMAs ride the
                    # gpsimd queue (25ns issue vs 565ns on SP)
                    tx = work.tile([rows, Wl], f16, tag="tx", name=f"tx_{t}")
                    V.tensor_mul(tx[:, :], txc[:, :], qe[:, 1:Wl + 1])
                    bounce = dramp.tile([2, Wl], f16, tag="bounce",
                                        name=f"bounce_{t}")
                    nc.gpsimd.dma_start(out=bounce[0:2, :],
                                        in_=tx[0:rows:rows - 1, :])
                    gsb = work.tile([GR, Wl], f16, tag="gsb", name=f"gsb_{t}")
                    for gi, rg in enumerate(rg_list):
                        gw = 2 * len(rg[0])
                        gkw = ({"addr_space": "Shared"}
                               if cc_mode == "ag8" else {})
                        gath = dramp.tile([gw, Wl], f16, tag=f"gath{gi}",
                                          name=f"gath{gi}_{t}", **gkw)
                        nc.gpsimd.collective_compute(
                            "AllGather", mybir.AluOpType.bypass,
                            ins=[bounce.opt()], outs=[gath.opt()],
                            replica_groups=rg,
                        )
                        nc.gpsimd.dma_start(out=gsb[gi * gw:(gi + 1) * gw, :],
                                            in_=gath[:, :])

            out_sb = coef.tile([rows, Wl], f32, name="out_sb")
            nc.vector.tensor_mul(out_sb[:, :], qe[:, 1:Wl + 1], rsl[:, :])
            nc.sync.dma_start(out=pout_d[:, :], in_=out_sb[:, :])

    nc.finalize()
    return nc


def _get_nc(n_cores, time_steps, nx, ny):
    key = (n_cores, time_steps, nx, ny)
    if key not in _cached:
        _cached[key] = _build(n_cores, time_steps, nx, ny)
    return _cached[key]


def kernel(u, f, time_steps):
    from concourse.bass_utils import run_bass_kernel_spmd

    u = np.asarray(u)
    f = np.asarray(f)
    ts = int(time_steps)
    N = u.shape[0]
    n_cores = NCORES
    nc = _get_nc(n_cores, ts, N, u.shape[1])
    in_maps = _host_inputs(u, f, n_cores, ts)
    res = run_bass_kernel_spmd(nc, in_maps, list(range(n_cores))).results
    interior = np.concatenate([r["pout"] for r in res], axis=0)
    h = 1.0 / (N - 1)
    xs = (np.arange(N, dtype=np.float64) * h).astype(np.float32)
    out = np.empty((N, N), dtype=np.float32)
    out[:, 1:N - 1] = interior
    out[:, 0] = xs
    out[:, N - 1] = 1.0 - xs
    return out


# revision 20
# speedup vs baseline: 2.1678x; 1.3827x over previous
"""Trainium2 Bass kernel for GroundwaterModel Jacobi pseudo-timestepping.

Solves 100 Jacobi steps of -div(exp(u) grad p) = f on a [1024,1024] grid,
sharded row-wise (x) across 8 NeuronCores with a 1-row halo exchange per
step (AllGather of pre-weighted boundary rows).

Math: with D = 2*eu + eu_xm + eu_ym (Jacobi diagonal), substitute
q = lam * sqrt(D) * p.  The update becomes

  q'[i,k] = bxu[i,k] q[i+1,k] + bxu[i-1,k] q[i-1,k]
          + by[i,k] q[i,k+1] + by[i,k-1] q[i,k-1] + c[i,k]

with bxu[i,k] = eu[i,j]*rs[i,j]*rs[i+1,j], by[i,k] = eu[i,j]*rs[i,j]*rs[i,j+1],
rs = 1/sqrt(D), c = lam*h^2*f*rs (+ Dirichlet fold at the two y-boundary
columns, Neumann folds at the x edges).  All coefficients are precomputed
on the host in fp64 and shipped as fp16; the iteration state q and the four
shift products run in fp16 on the DVE (2x mode), the partition-dim shifts
and halo injection accumulate in fp32 PSUM via fp16 matmuls (1 cycle/row),
and the per-step inter-core halo is an AllGather of the two boundary
products, issued at the top of each step so it overlaps the interior work.
"""

import numpy as np

GRID = 1024
NCORES = 8
P = 128          # rows per core = SBUF partitions
W = GRID - 2     # computed interior columns j=1..GRID-2
LAM = 1024.0     # q scaling to keep fp16 constants out of the subnormal range
CC_MODE = "ag8"  # "ag8": one 8-core AllGather; "pair": two 2-core AllGathers (hangs NRT)

_cached = {}


def _host_inputs(u, f, n_cores, time_steps, cc_mode=CC_MODE):
    """Per-core input dicts. All per-core variation lives in data."""
    N = u.shape[0]
    h = 1.0 / (N - 1)
    rows = N // n_cores
    Wl = N - 2

    eu = np.exp(u.astype(np.float64))
    eu_xm = np.concatenate([eu[:1, :], eu[:-1, :]], axis=0)
    eu_ym = np.concatenate([eu[:, :1], eu[:, :-1]], axis=1)
    D = 2.0 * eu + eu_xm + eu_ym
    rs = 1.0 / np.sqrt(D)
    h2f = (h * h) * f.astype(np.float64)
    xs = np.arange(N, dtype=np.float64) * h
    bc0 = xs
    bc1 = 1.0 - xs
    j = np.arange(1, N - 1)

    # x-coupling (i,j)<->(i+1,j); row N-1 replaced by the Neumann bottom fold
    bxu = np.zeros((N, Wl))
    bxu[:-1, :] = eu[:-1, j] * rs[:-1, j] * rs[1:, j]
    b_top = eu[0, j] * rs[0, j] * rs[0, j]
    b_bot = eu[N - 1, j] * rs[N - 1, j] * rs[N - 1, j]
    # y-coupling (i,j)<->(i,j+1); column Wl-1 is Dirichlet-folded -> 0
    by = np.zeros((N, Wl))
    by[:, :-1] = eu[:, j[:-1]] * rs[:, j[:-1]] * rs[:, j[:-1] + 1]
    # constants
    c0 = h2f[:, j] * rs[:, j]
    c = c0.copy()
    c[:, 0] += eu_ym[:, 1] * bc0 * rs[:, 1]
    c[:, -1] += eu[:, N - 2] * bc1 * rs[:, N - 2]
    c = LAM * c
    c0 = LAM * c0

    f16 = np.float16
    in_maps = []
    for cidx in range(n_cores):
        r0 = cidx * rows
        A = bxu[r0:r0 + rows].copy()
        if cidx == n_cores - 1:
            A[-1] = b_bot
        Ad = np.zeros((rows, Wl))
        Ad[1:] = bxu[r0:r0 + rows - 1]
        Ad[0] = b_top if cidx == 0 else bxu[r0 - 1]
        By = by[r0:r0 + rows]
        Byd = np.zeros((rows, Wl))
        Byd[:, 1:] = By[:, :-1]

        supT = np.zeros((rows, rows), dtype=f16)
        for i in range(rows - 1):
            supT[i + 1, i] = 1.0          # out[i] += u1[i+1]
        if cidx == 0:
            supT[0, 0] = 1.0              # Neumann top edge via u1[0]
        sdnT = np.zeros((rows, rows), dtype=f16)
        for i in range(1, rows):
            sdnT[i - 1, i] = 1.0          # out[i] += u2[i-1]
        if cidx == n_cores - 1:
            sdnT[rows - 1, rows - 1] = 1.0  # Neumann bottom edge via u2[last]

        if cc_mode == "ag8":
            GR = 2 * n_cores
            eT = np.zeros((GR, rows), dtype=f16)
            if cidx > 0:
                eT[2 * cidx - 1, 0] = 1.0       # prev core's tx2 -> my row 0
            if cidx < n_cores - 1:
                eT[2 * cidx + 2, rows - 1] = 1.0  # next core's tx1 -> my last row
        else:
            # two 2-core AllGathers; gsb rows 0-3 = CC1 pair, 4-7 = CC2 pair,
            # each pair in ascending rank order as [lo_tx1, lo_tx2, hi_tx1, hi_tx2]
            eT = np.zeros((8, rows), dtype=f16)
            if cidx % 2 == 0:
                if cidx + 1 < n_cores:
                    eT[2, rows - 1] = 1.0   # CC1 partner is next: its tx1
                if cidx > 0:
                    eT[4 + 1, 0] = 1.0      # CC2 partner is prev: its tx2
            else:
                eT[1, 0] = 1.0              # CC1 partner is prev: its tx2
                if cidx + 1 < n_cores:
                    eT[4 + 2, rows - 1] = 1.0  # CC2 partner is next: its tx1

        txc = np.zeros((rows, Wl))
        txc[0] = Ad[0]          # tx1: product sent to prev core
        txc[-1] = A[-1]         # tx2: product sent to next core
        # halo-correction coefficients: my row-0 halo total is prev's
        # partial + Ad[0] * (my own T1 from two steps ago); zero on edges
        cpcx = np.zeros((rows, Wl))
        if cidx > 0:
            cpcx[0] = Ad[0]
        if cidx < n_cores - 1:
            cpcx[-1] = A[-1]
        ident = np.eye(rows, dtype=f16)
        in_maps.append({
            "cpcx": cpcx.astype(f16), "ident": ident,
            "A": A.astype(f16), "Ad": Ad.astype(f16), "txc": txc.astype(f16),
            "By": By.astype(f16), "Byd": Byd.astype(f16),
            "cp": c[r0:r0 + rows].astype(f16),
            "cp0": c0[r0:r0 + rows].astype(f16),
            "supT": supT, "sdnT": sdnT, "eT": eT,
            "rsl": (rs[r0:r0 + rows, j] / LAM).astype(np.float32),
        })
    return in_maps


def _build(n_cores, time_steps, nx, ny, cc_mode=CC_MODE):
    import concourse.bass as bass
    import concourse.bacc as bacc
    import concourse.mybir as mybir
    from concourse.tile import TileContext

    f32 = mybir.dt.float32
    f16 = mybir.dt.float16
    G = ny
    Wl = G - 2
    rows = nx // n_cores
    GR = 2 * n_cores if cc_mode == "ag8" else 8

    nc = bacc.Bacc(
        "TRN2",
        target_bir_lowering=False,
        debug=False,
        num_devices=n_cores,
    )
    dp = nc.declare_dram_parameter
    A_d = dp("A", [rows, Wl], f16, isOutput=False)
    Ad_d = dp("Ad", [rows, Wl], f16, isOutput=False)
    txc_d = dp("txc", [rows, Wl], f16, isOutput=False)
    cpcx_d = dp("cpcx", [rows, Wl], f16, isOutput=False)
    ident_d = dp("ident", [rows, rows], f16, isOutput=False)
    By_d = dp("By", [rows, Wl], f16, isOutput=False)
    Byd_d = dp("Byd", [rows, Wl], f16, isOutput=False)
    cp_d = dp("cp", [rows, Wl], f16, isOutput=False)
    cp0_d = dp("cp0", [rows, Wl], f16, isOutput=False)
    supT_d = dp("supT", [rows, rows], f16, isOutput=False)
    sdnT_d = dp("sdnT", [rows, rows], f16, isOutput=False)
    eT_d = dp("eT", [GR, rows], f16, isOutput=False)
    rsl_d = dp("rsl", [rows, Wl], f32, isOutput=False)
    pout_d = dp("pout", [rows, Wl], f32, isOutput=True)

    if cc_mode == "ag8":
        rg_list = [[list(range(n_cores))]]
    else:
        rg_list = [
            [[0, 1], [2, 3], [4, 5], [6, 7]],
            [[0, 7], [1, 2], [3, 4], [5, 6]],
        ]

    with TileContext(nc) as tc:
        with (
            tc.tile_pool(name="coef", bufs=1) as coef,
            tc.tile_pool(name="work", bufs=2) as work,
            tc.tile_pool(name="qp", bufs=2, space="PSUM") as qp,
            tc.tile_pool(name="dramp", bufs=2, space="DRAM") as dramp,
        ):
            # ---- persistent tiles (coefficients + state) ----
            A = coef.tile([rows, Wl], f16, name="A_t")
            Ad = coef.tile([rows, Wl], f16, name="Ad_t")
            txc = coef.tile([rows, Wl], f16, name="txc_t")
            cpcx = coef.tile([rows, Wl], f16, name="cpcx_t")
            ident = coef.tile([rows, rows], f16, name="ident_t")
            By = coef.tile([rows, Wl], f16, name="By_t")
            Byd = coef.tile([rows, Wl], f16, name="Byd_t")
            cp = coef.tile([rows, Wl], f16, name="cp_t")
            cp0 = coef.tile([rows, Wl], f16, name="cp0_t")
            supT = coef.tile([rows, rows], f16, name="supT_t")
            sdnT = coef.tile([rows, rows], f16, name="sdnT_t")
            eT = coef.tile([GR, rows], f16, name="eT_t")
            rsl = coef.tile([rows, Wl], f32, name="rsl_t")
            qe = coef.tile([rows, Wl + 2], f16, name="qe")  # pad cols 0, Wl+1
            for t_, d_ in ((A, A_d), (Ad, Ad_d), (txc, txc_d),
                           (cpcx, cpcx_d), (ident, ident_d),
                           (By, By_d), (Byd, Byd_d),
                           (cp, cp_d), (cp0, cp0_d), (supT, supT_d),
                           (sdnT, sdnT_d), (eT, eT_d), (rsl, rsl_d)):
                nc.sync.dma_start(out=t_[:, :], in_=d_[:, :])

            B0 = 512                     # PSUM bank split
            banks = [(0, B0), (B0, Wl)]
            V = nc.vector
            mm = nc.tensor.matmul

            nc.vector.memset(qe[:, 0:1], 0.0)
            nc.vector.memset(qe[:, Wl + 1:Wl + 2], 0.0)

            def send_cc(t, tx):
                # bounce -> AllGather -> gsb; DMAs ride the gpsimd queue
                # (25ns issue vs 565ns on SP)
                bounce = dramp.tile([2, Wl], f16, tag="bounce",
                                    name=f"bounce_{t}")
                nc.gpsimd.dma_start(out=bounce[0:2, :],
                                    in_=tx[0:rows:rows - 1, :])
                gsb = work.tile([GR, Wl], f16, tag="gsb", name=f"gsb_{t}")
                for gi, rg in enumerate(rg_list):
                    gw = 2 * len(rg[0])
                    gkw = {"addr_space": "Shared"} if cc_mode == "ag8" else {}
                    gath = dramp.tile([gw, Wl], f16, tag=f"gath{gi}",
                                      name=f"gath{gi}_{t}", **gkw)
                    nc.gpsimd.collective_compute(
                        "AllGather", mybir.AluOpType.bypass,
                        ins=[bounce.opt()], outs=[gath.opt()],
                        replica_groups=rg,
                    )
                    # gsb load rides SP: on the gpsimd queue it would block
                    # the NEXT step's bounce/trigger behind this collective
                    nc.sync.dma_start(out=gsb[gi * gw:(gi + 1) * gw, :],
                                      in_=gath[:, :])
                return gsb

            # Pipelined exchange: each step sends txc*L computed from local
            # state only (L = y-terms + cp + local x-terms), so step t's
            # AllGather launches without waiting for step t-1's to land.
            # The receiver reconstructs the exact total product:
            #   T_halo(t-1) = gsb(t-1) partials + cpcx * T(t-2)
            # where T(t) = txc * q(t) is the true boundary product, kept
            # locally.  Exact by linearity of the update in the halo term.
            gsb = None
            Tm2 = Tm1 = None   # T tiles from two steps back / one step back
            for t in range(1, time_steps + 1):
                if t == 1:
                    V.tensor_copy(qe[:, 1:Wl + 1], cp0[:, :])  # q_1 = lam*h2f*rs
                    tx = work.tile([rows, Wl], f16, tag="tx", name="tx_1")
                    V.tensor_mul(tx[:, :], txc[:, :], qe[:, 1:Wl + 1])
                    gsb = send_cc(t, tx)
                    # allocate T_0 first so the 2-buffer cycle alternates
                    # T_even/T_odd and T_t only ever reuses T_{t-2}'s buffer
                    Tm2 = work.tile([rows, Wl], f16, tag="T", name="T_0")
                    V.memset(Tm2[:, :], 0.0)
                    Tm1 = work.tile([rows, Wl], f16, tag="T", name="T_1")
                    V.tensor_copy(Tm1[:, :], tx[:, :])  # q_1 has no halo
                    continue

                # products for the x-shifts (PE) and y-shifts (free-dim)
                u1 = work.tile([rows, Wl], f16, tag="u1", name=f"u1_{t}")
                u2 = work.tile([rows, Wl], f16, tag="u2", name=f"u2_{t}")
                y1 = work.tile([rows, Wl], f16, tag="y1", name=f"y1_{t}")
                y2 = work.tile([rows, Wl], f16, tag="y2", name=f"y2_{t}")
                V.tensor_mul(u1[:, :], Ad[:, :], qe[:, 1:Wl + 1])
                V.tensor_mul(u2[:, :], A[:, :], qe[:, 1:Wl + 1])
                V.tensor_mul(y1[:, :], By[:, :], qe[:, 2:Wl + 2])
                V.tensor_mul(y2[:, :], Byd[:, :], qe[:, 0:Wl])
                # PE group 1: local x-terms + cp
                psx = qp.tile([rows, Wl], f32, tag="psx", name=f"psx_{t}")
                for lo, hi in banks:
                    mm(psx[:, lo:hi], supT[:, :], u1[:, lo:hi],
                       start=True, stop=False)
                for lo, hi in banks:
                    mm(psx[:, lo:hi], sdnT[:, :], u2[:, lo:hi],
                       start=False, stop=False)
                for lo, hi in banks:
                    mm(psx[:, lo:hi], ident[:, :], cp[:, lo:hi],
                       start=False, stop=True)
                a1 = work.tile([rows, Wl], f16, tag="a1", name=f"a1_{t}")
                V.tensor_add(a1[:, :], y1[:, :], y2[:, :])
                # L: this step's update minus the halo term (local only)
                L = work.tile([rows, Wl], f16, tag="L", name=f"L_{t}")
                V.tensor_add(L[:, :], a1[:, :], psx[:, :])
                if t < time_steps:
                    # launch this step's exchange from L -- does not wait on
                    # the previous collective
                    tx = work.tile([rows, Wl], f16, tag="tx", name=f"tx_{t}")
                    V.tensor_mul(tx[:, :], txc[:, :], L[:, :])
                    gsb_next = send_cc(t, tx)
                else:
                    gsb_next = None
                # assemble the exact halo for THIS step: partials from the
                # previous collective + correction from T(t-2)
                corr = work.tile([rows, Wl], f16, tag="corr",
                                 name=f"corr_{t}")
                V.tensor_mul(corr[:, :], cpcx[:, :], Tm2[:, :])
                psh = qp.tile([rows, Wl], f32, tag="psh", name=f"psh_{t}")
                for lo, hi in banks:
                    mm(psh[:, lo:hi], eT[:, :], gsb[:, lo:hi],
                       start=True, stop=False)
                for lo, hi in banks:
                    mm(psh[:, lo:hi], ident[:, :], corr[:, lo:hi],
                       start=False, stop=True)
                V.tensor_add(qe[:, 1:Wl + 1], L[:, :], psh[:, :])
                gsb = gsb_next
                if t <= time_steps - 2:
                    Tnew = work.tile([rows, Wl], f16, tag="T", name=f"T_{t}")
                    V.tensor_mul(Tnew[:, :], txc[:, :], qe[:, 1:Wl + 1])
                    Tm2, Tm1 = Tm1, Tnew
                else:
                    Tm2, Tm1 = Tm1, None

            out_sb = coef.tile([rows, Wl], f32, name="out_sb")
            nc.vector.tensor_mul(out_sb[:, :], qe[:, 1:Wl + 1], rsl[:, :])
            nc.sync.dma_start(out=pout_d[:, :], in_=out_sb[:, :])

    nc.finalize()
    return nc


def _get_nc(n_cores, time_steps, nx, ny):
    key = (n_cores, time_steps, nx, ny)
    if key not in _cached:
        _cached[key] = _build(n_cores, time_steps, nx, ny)
    return _cached[key]


def kernel(u, f, time_steps):
    from concourse.bass_utils import run_bass_kernel_spmd

    u = np.asarray(u)
    f = np.asarray(f)
    ts = int(time_steps)
    N = u.shape[0]
    n_cores = NCORES
    nc = _get_nc(n_cores, ts, N, u.shape[1])
    in_maps = _host_inputs(u, f, n_cores, ts)
    res = run_bass_kernel_spmd(nc, in_maps, list(range(n_cores))).results
    interior = np.concatenate([r["pout"] for r in res], axis=0)
    h = 1.0 / (N - 1)
    xs = (np.arange(N, dtype=np.float64) * h).astype(np.float32)
    out = np.empty((N, N), dtype=np.float32)
    out[:, 1:N - 1] = interior
    out[:, 0] = xs
    out[:, N - 1] = 1.0 - xs
    return out
